# revision 9
# baseline (speedup 1.0000x reference)
"""CPSF fused codebook kernel for Trainium2 (8 NeuronCores, codebook-parallel).

Sharding: M (codebook, 4096) split 8 ways -> 512 entries/core; every core sees
all B=2048 queries. Host sums the 8 partial [B,S] outputs.

v3 design vs the K=8 quadrature baseline:
  The Gauss-Legendre sum over ray parameter t is a quadrature of
  int_0^1 exp(-A (t - mu)^2) dt = sqrt(pi)/(2 sqA) [erf(sqA(1-mu)) + erf(sqA mu)]
  (the integrand's exponent is quadratic in t). GL8 is ~exact for these smooth
  integrands, so the kernel evaluates the closed form: per (b,m) the 9 Exp ACT
  passes become 2 Erf + 1 Exp.

  wgt = exp(Cx*x_ip^2 + Gd*y_ip^2 + co*(q0 + dist_d) + ln a + ln pref) * esum
  assembled as: PE accumulates F3 = co-terms (fp32r matmuls) then adds the
  squares into the same PSUM bank via identity-matmuls (fp16 moving, diag(sgn)
  weights carry the y-sign); Exp reads the finished PSUM directly with pconst
  as its per-partition bias. Squares: x^2 on ACT (Square reads PSUM at full
  rate), y^2 on DVE (cast PSUM->fp16 at copy rate + packed 2x fp16 multiply).

  Matmuls run in float32r (1 cyc/row vs fp32's 4 — measured ~2^-13 accurate on
  HW, plenty inside the 2e-2 gate). `that` matmul + elementwise tail in bf16.
  ACT table sets force two phases: Erf (sigmoid set) first, one table switch,
  then Exp (exp set; Square is in both sets).
"""

import numpy as np

B, M, N, S, K = 2048, 4096, 64, 64, 8
EPS = 1e-3
NCORES = 8
ML = M // NCORES          # 512 codebook entries per core
MT = ML // 128            # 4 m-tiles per core
H = 1024                  # b-half
Q = 512                   # b-quarter
f32 = np.float32

_CACHE = {}
SQX_DVE = (2, 3)          # m-tiles whose x^2 runs on DVE instead of ACT
ESUM_DMA = False          # esum += e2 via gpsimd software-DGE DMA accum


def _prep(z_re, z_im, d_re, d_im, zj_re, zj_im, dj_re, dj_im,
          That_re, That_im, alpha, sig_par, sig_perp):
    """Host-side packing: fp64 exact, cast to fp32/bf16/fp16 at the end."""
    from ml_dtypes import bfloat16
    x64 = lambda a: np.asarray(a, np.float64)
    zr, zi, dr, di = map(x64, (z_re, z_im, d_re, d_im))
    zjr, zji, djr, dji = map(x64, (zj_re, zj_im, dj_re, dj_im))

    sp2 = x64(sig_par) ** 2 + EPS
    so2 = x64(sig_perp) ** 2 + EPS
    G = -0.5 / sp2
    co = -0.5 / so2
    Gd = G - co
    dd2 = (djr**2 + dji**2).sum(-1)
    c_re = (djr * zjr + dji * zji).sum(-1)
    c_im = (djr * zji - dji * zjr).sum(-1)
    nzj = (zjr**2 + zji**2).sum(-1)
    nz = (zr**2 + zi**2).sum(-1)
    nd = (dr**2 + di**2).sum(-1)
    nz_m, nd_m = nz.mean(), nd.mean()

    A_ = -dd2 * (Gd * dd2 + co)
    sqA = np.sqrt(A_)
    Cx = -co / dd2
    sx = np.sqrt(Cx)
    sy = np.sqrt(np.abs(Gd))
    psgn = np.where(Gd >= 0, 1.0, -1.0)
    lnal = np.log(np.maximum(x64(alpha), 1e-300))
    pconst = (co * (nzj + dd2 + nz_m + nd_m) + Cx * c_re**2 + Gd * c_im**2
              + lnal + np.log(np.sqrt(np.pi) / (2.0 * sqA)))
    e2s = sqA / (dd2 * sx)
    e2b = -sqA * c_re / dd2
    e1s = -e2s
    e1b = sqA + sqA * c_re / dd2

    djx = np.concatenate([djr.T, dji.T], 0)      # [128, M]
    djy = np.concatenate([-dji.T, djr.T], 0)
    Wxs = sx[None, :] * djx                      # xs = sx * x_pe
    Wys = sy[None, :] * djy                      # ys = sy * y_pe
    Wz = (-2.0 * co[None, :] * np.concatenate([zjr.T, zji.T], 0)
          + (2.0 * co * c_re / dd2)[None, :] * djx
          + (-2.0 * Gd * c_im)[None, :] * djy)
    Wd = -2.0 * co[None, :] * np.concatenate([djr.T, dji.T], 0)
    f3c = np.stack([co, co])                     # [2, M]
    rhsc = np.stack([nz - nz_m, nd - nd_m])      # [2, B] centered

    nt = M // 128
    pk = np.empty((128, nt * 512), np.float64)
    for j in range(nt):
        ms = slice(j * 128, (j + 1) * 128)
        pk[:, j * 512 + 0:j * 512 + 128] = Wxs[:, ms]
        pk[:, j * 512 + 128:j * 512 + 256] = Wys[:, ms]
        pk[:, j * 512 + 256:j * 512 + 384] = Wz[:, ms]
        pk[:, j * 512 + 384:j * 512 + 512] = Wd[:, ms]

    # identity + per-tile diag(sign) weights for the PSUM square-accumulate
    idw = np.zeros((128, (nt + 1) * 128), np.float64)
    idw[:, 0:128] = np.eye(128)
    for j in range(nt):
        idw[:, (j + 1) * 128:(j + 2) * 128] = np.diag(psgn[j * 128:(j + 1) * 128])

    # per-m scalars, [128, nt] each: pconst, e1s, e1b, e2s, e2b
    P = lambda a: a.reshape(nt, 128).T
    params = np.concatenate([P(pconst), P(e1s), P(e1b),
                             P(e2s), P(e2b)], axis=1)   # [128, 5*nt]

    that2 = np.concatenate([x64(That_re), x64(That_im)], 1)  # [M, 128]

    c = lambda a: np.ascontiguousarray(a, dtype=f32)
    return dict(pk=c(pk), f3c=c(f3c), params=c(params),
                idw=np.ascontiguousarray(idw.astype(np.float16)),
                that2=np.ascontiguousarray(that2.astype(bfloat16)),
                zst=c(np.concatenate([zr.T, zi.T], 0)),
                dst=c(np.concatenate([dr.T, di.T], 0)), rhsc=c(rhsc))


def _core_slices(p, cid):
    """Per-core in_map from the full packed arrays (m-sharded)."""
    nt = M // 128
    jt = slice(cid * MT * 512, (cid + 1) * MT * 512)         # pk cols
    ms = slice(cid * ML, (cid + 1) * ML)
    kc = np.concatenate([np.arange(k * nt + cid * MT,
                                   k * nt + (cid + 1) * MT)
                         for k in range(5)])
    ic = np.concatenate([np.arange(128),
                         np.arange((1 + cid * MT) * 128,
                                   (1 + (cid + 1) * MT) * 128)])
    cc = np.ascontiguousarray
    return {"pk": cc(p["pk"][:, jt]), "f3c": cc(p["f3c"][:, ms]),
            "that2": cc(p["that2"][ms, :]),
            "params": cc(p["params"][:, kc]),
            "idw": cc(p["idw"][:, ic]),
            "zst": p["zst"], "dst": p["dst"], "rhsc": p["rhsc"]}


def _device_maps(maps):
    return maps


def _emulate_core(m):
    """Numpy emulation of one core's device program (f32/f16-ish)."""
    from scipy.special import erf as serf
    zst, dst, rhsc = (np.float64(m["zst"]), np.float64(m["dst"]),
                      np.float64(m["rhsc"]))
    pr = np.float64(m["params"])
    pconst = pr[:, 0:MT]
    e1s, e1b = pr[:, MT:2*MT], pr[:, 2*MT:3*MT]
    e2s, e2b = pr[:, 3*MT:4*MT], pr[:, 4*MT:5*MT]
    idw = np.float64(m["idw"])
    t_acc = np.zeros((128, B), f32)
    for j in range(MT):
        pkj = np.float64(m["pk"][:, j * 512:(j + 1) * 512])
        xw, yw = pkj[:, 0:128], pkj[:, 128:256]
        zw, dw = pkj[:, 256:384], pkj[:, 384:512]
        isgn = idw[:, (j + 1) * 128:(j + 2) * 128]
        xs = (xw.T @ zst).astype(f32)
        ys = (yw.T @ zst).astype(f32)
        xx = (xs * xs).astype(f32).astype(np.float16).astype(np.float64)
        ysh = ys.astype(np.float16).astype(np.float64)
        yy = (ysh * ysh).astype(np.float16).astype(np.float64)
        F3 = (zw.T @ zst + dw.T @ dst
              + np.float64(m["f3c"][:, j*128:(j+1)*128]).T @ rhsc
              + xx + isgn.T @ yy).astype(f32)
        e1 = serf(e1s[:, j:j+1] * xs + e1b[:, j:j+1]).astype(f32)
        e2 = serf(e2s[:, j:j+1] * xs + e2b[:, j:j+1]).astype(f32)
        esum = (e1 + e2).astype(f32)
        EB = np.exp(F3 + pconst[:, j:j+1]).astype(f32)
        wgt = (EB * esum).astype(f32)
        that_t = np.float64(m["that2"][j * 128:(j + 1) * 128, :])
        t_acc += (that_t.T @ wgt).astype(f32)
    return t_acc


def _build_bass():
    import concourse.bacc as bacc
    import concourse.mybir as mybir
    from concourse import tile

    dt = mybir.dt
    AF = mybir.ActivationFunctionType
    AO = mybir.AluOpType
    nc = bacc.Bacc("TRN2", target_bir_lowering=False, debug=False)

    r32 = dt.float32r
    bf = dt.bfloat16
    fh = dt.float16
    dram = {}
    for name, shape, dty in [("zst", [128, B], r32), ("dst", [128, B], r32),
                             ("rhsc", [2, B], r32),
                             ("pk", [128, MT * 512], r32),
                             ("f3c", [2, ML], r32),
                             ("idw", [128, (MT + 1) * 128], fh),
                             ("that2", [ML, 128], bf),
                             ("params", [128, 5 * MT], dt.float32)]:
        dram[name] = nc.dram_tensor(name, shape, dty, kind="ExternalInput")
    tout = nc.dram_tensor("tout", [128, B], dt.float32, kind="ExternalOutput")

    with tile.TileContext(nc) as tc:
        with tc.tile_pool(name="const", bufs=1) as cpool:
            params = cpool.tile([128, 5 * MT], dt.float32)
            pconst = params[:, 0:MT]
            e1s = params[:, MT:2 * MT]
            e1b = params[:, 2 * MT:3 * MT]
            e2s = params[:, 3 * MT:4 * MT]
            e2b = params[:, 4 * MT:5 * MT]
            rhsc = cpool.tile([2, B], r32)
            f3c_all = cpool.tile([2, ML], r32)
            that_all = cpool.tile([128, MT * 128], bf)
            idw = cpool.tile([128, (MT + 1) * 128], fh)
            warm = cpool.tile([128, 8], dt.float32)
            zst = cpool.tile([128, B], r32)
            dst = cpool.tile([128, B], r32)
            xxs = [cpool.tile([128, B], fh, name=f"xx{j}") for j in range(MT)]
            yys = [cpool.tile([128, B], fh, name=f"yy{j}") for j in range(MT)]
            esums = [cpool.tile([128, B], bf, name=f"esum{j}")
                     for j in range(MT)]
            ocp = cpool.tile([128, B], dt.float32)

            nc.sync.dma_start(params[:, :], dram["params"][:, :])
            # fire the ACT erf (sigmoid-set) table-load ASAP
            nc.scalar.activation(warm[:, :], params[:, 0:8], AF.Erf)

            with (
                tc.tile_pool(name="lhs", bufs=5) as lpool,
                tc.tile_pool(name="work", bufs=3) as wpool,
                tc.tile_pool(name="gp", bufs=2) as gpool,
            ):
                pks = [lpool.tile([128, 512], r32, tag="pk",
                                  name=f"pk{j}") for j in range(MT)]
                nc.sync.dma_start(zst[:, 0:Q], dram["zst"][:, 0:Q])
                nc.sync.dma_start(zst[:, Q:2*Q], dram["zst"][:, Q:2*Q])
                nc.sync.dma_start(pks[0][:, :], dram["pk"][:, 0:512])
                nc.sync.dma_start(zst[:, 2*Q:3*Q], dram["zst"][:, 2*Q:3*Q])
                nc.sync.dma_start(zst[:, 3*Q:4*Q], dram["zst"][:, 3*Q:4*Q])
                for j in range(1, MT):
                    nc.sync.dma_start(pks[j][:, :],
                                      dram["pk"][:, j * 512:(j + 1) * 512])
                for i in range(2):
                    hs = slice(i * H, (i + 1) * H)
                    nc.sync.dma_start(dst[:, hs], dram["dst"][:, hs])
                nc.sync.dma_start(rhsc[:, :], dram["rhsc"][:, :])
                nc.sync.dma_start(f3c_all[:, :], dram["f3c"][:, :])
                nc.sync.dma_start(idw[:, :], dram["idw"][:, :])
                nc.sync.dma_start(
                    that_all[:, :].rearrange("p (j c) -> p j c", j=MT),
                    dram["that2"][:, :].rearrange("(j p) c -> p j c", p=128))

                # ---- phase A: erf + squares (sigmoid table set) ----
                with (
                    tc.tile_pool(name="xps", bufs=2, space="PSUM") as xpool,
                    tc.tile_pool(name="yps", bufs=1, space="PSUM") as ypool,
                    tc.tile_pool(name="wps", bufs=1, space="PSUM") as spool,
                ):
                    scr = spool.tile([128, 128], dt.float32)
                    for _ in range(24):
                        nc.tensor.matmul(scr[0:20, 0:5 * MT],
                                         params[:, 0:20], params[:, :],
                                         start=True, stop=True)
                    for j in range(MT):
                        pk_t = pks[j]
                        xw = pk_t[:, 0:128]
                        yw = pk_t[:, 128:256]
                        for h in range(2):
                            hs = slice(h * H, (h + 1) * H)
                            xs = xpool.tile([128, H], dt.float32, tag="xs",
                                            name=f"xs{j}_{h}")
                            ys = ypool.tile([128, H], dt.float32, tag="ys",
                                            name=f"ys{j}_{h}")
                            for q in range(2):
                                ql = slice(q * Q, (q + 1) * Q)
                                qg = slice(h * H + q * Q, h * H + (q + 1) * Q)
                                nc.tensor.matmul(xs[:, ql], xw, zst[:, qg],
                                                 start=True, stop=True)
                            for q in range(2):
                                ql = slice(q * Q, (q + 1) * Q)
                                qg = slice(h * H + q * Q, h * H + (q + 1) * Q)
                                nc.tensor.matmul(ys[:, ql], yw, zst[:, qg],
                                                 start=True, stop=True)

                            e2 = wpool.tile([128, H], bf, tag="e2",
                                            name=f"e2_{j}_{h}")
                            nc.scalar.activation(esums[j][:, hs], xs[:, :],
                                                 AF.Erf,
                                                 bias=e1b[:, j:j + 1],
                                                 scale=e1s[:, j:j + 1])
                            nc.scalar.activation(e2[:, :], xs[:, :], AF.Erf,
                                                 bias=e2b[:, j:j + 1],
                                                 scale=e2s[:, j:j + 1])
                            if j in SQX_DVE:
                                xh = wpool.tile([128, H], fh, tag="xh",
                                                name=f"xh{j}_{h}")
                                nc.vector.tensor_copy(xh[:, :], xs[:, :])
                                nc.vector.tensor_mul(xxs[j][:, hs],
                                                     xh[:, :], xh[:, :])
                            else:
                                nc.scalar.activation(xxs[j][:, hs], xs[:, :],
                                                     AF.Square)
                            yh = wpool.tile([128, H], fh, tag="yh",
                                            name=f"yh{j}_{h}")
                            nc.vector.tensor_copy(yh[:, :], ys[:, :])
                            nc.vector.tensor_mul(yys[j][:, hs],
                                                 yh[:, :], yh[:, :])
                            if ESUM_DMA:
                                nc.gpsimd.dma_start(esums[j][:, hs],
                                                    e2[:, :],
                                                    accum_op=AO.add)
                            else:
                                nc.vector.tensor_add(esums[j][:, hs],
                                                     esums[j][:, hs],
                                                     e2[:, :])

                # ---- phase B: F3 + exp + that (exp table set) ----
                with (
                    tc.tile_pool(name="fps", bufs=2, space="PSUM") as fpool,
                    tc.tile_pool(name="tps", bufs=1, space="PSUM") as tpool,
                ):
                    taccs = [tpool.tile([128, H], dt.float32, name=f"tacc{h}")
                             for h in range(2)]
                    ix = idw[:, 0:128]
                    for j in range(MT):
                        pk_t = pks[j]
                        zw = pk_t[:, 256:384]
                        dw = pk_t[:, 384:512]
                        f3c_t = f3c_all[:, j * 128:(j + 1) * 128]
                        isgn = idw[:, (j + 1) * 128:(j + 2) * 128]
                        that_t = that_all[:, j * 128:(j + 1) * 128]
                        for h in range(2):
                            hs = slice(h * H, (h + 1) * H)
                            F3 = fpool.tile([128, H], dt.float32, tag="f3",
                                            name=f"f3{j}_{h}")
                            for q in range(2):
                                ql = slice(q * Q, (q + 1) * Q)
                                qg = slice(h * H + q * Q, h * H + (q + 1) * Q)
                                nc.tensor.matmul(F3[:, ql], zw, zst[:, qg],
                                                 start=True, stop=False)
                                nc.tensor.matmul(F3[:, ql], dw, dst[:, qg],
                                                 start=False, stop=False)
                                nc.tensor.matmul(F3[:, ql], f3c_t,
                                                 rhsc[:, qg],
                                                 start=False, stop=False)
                                nc.tensor.matmul(F3[:, ql], ix,
                                                 xxs[j][:, qg],
                                                 start=False, stop=False)
                                nc.tensor.matmul(F3[:, ql], isgn,
                                                 yys[j][:, qg],
                                                 start=False, stop=True)
                            EB = gpool.tile([128, H], bf, tag="EB",
                                            name=f"EB{j}_{h}")
                            wgt = gpool.tile([128, H], bf, tag="wgt",
                                             name=f"wgt{j}_{h}")
                            nc.scalar.activation(EB[:, :], F3[:, :], AF.Exp,
                                                 bias=pconst[:, j:j + 1])
                            nc.vector.tensor_mul(wgt[:, :], EB[:, :],
                                                 esums[j][:, hs])
                            for q in range(2):
                                ql = slice(q * Q, (q + 1) * Q)
                                nc.tensor.matmul(taccs[h][:, ql], that_t,
                                                 wgt[:, ql],
                                                 start=(j == 0),
                                                 stop=(j == MT - 1))
                    for h in range(2):
                        hs = slice(h * H, (h + 1) * H)
                        nc.vector.tensor_copy(ocp[:, hs], taccs[h][:, :])
                        nc.sync.dma_start(tout[:, hs], ocp[:, hs])

    nc.compile()
    return nc


def kernel(z_re, z_im, d_re, d_im, zj_re, zj_im, dj_re, dj_im,
           That_re, That_im, alpha, sig_par, sig_perp, _emulate=False):
    p = _prep(z_re, z_im, d_re, d_im, zj_re, zj_im, dj_re, dj_im,
              That_re, That_im, alpha, sig_par, sig_perp)
    maps = [_core_slices(p, c) for c in range(NCORES)]

    if _emulate:
        outs = [_emulate_core(m) for m in maps]
    else:
        from concourse.bass_utils import run_bass_kernel_spmd
        if "nc" not in _CACHE:
            _CACHE["nc"] = _build_bass()
        res = run_bass_kernel_spmd(_CACHE["nc"], _device_maps(maps),
                                   core_ids=list(range(NCORES)))
        outs = [res.results[c]["tout"] for c in range(NCORES)]

    full = np.zeros((128, B), np.float64)
    for o in outs:
        full += o.astype(np.float64)
    full = full.astype(f32).T                   # [B, 128]
    return (full[:, :S] + 1j * full[:, S:]).astype(np.complex64)
